# revision 15
# baseline (speedup 1.0000x reference)
"""Trainium2 Bass kernel for nn_Attention (GroupNorm + 1x1-conv QKV + MHA + out-proj + residual).

Sharding: data-parallel over batch — b=8 maps one batch element per NeuronCore (8 cores).
Weights are broadcast to all cores. No collectives.

v2: all matmuls run as fp8e4 DoubleRow (0.5 cycles/row, two 128-row K-tiles
per pass = 4x the fp32r matmul throughput). Weights are pre-scaled x16 on the
host so their values sit in e4m3's normal range; the scale is compensated by
exact power-of-2 folds (exp scale /256, output epilogue x1/256). The d=64
attention matmuls get DoubleRow by zero-padding the second K-tile (PE cost
depends only on the output free size, so the zeros are free). The k-bias
cancels in softmax and the v-bias is folded into the out-proj bias (host), so
only the q-bias is applied on-device. GroupNorm's rsqrt runs on DVE via the
inverse-sqrt bit trick + 2 Newton steps, keeping ACT's table pinned to Exp
(Sqrt lives in a different ACT table; switching costs 2x 1.28us per pass).
ACT then runs only the 64 exp instructions and paces the pipeline.

Per-core pipeline (one batch element, x: [512 ch, 1024 spatial]):
  - GroupNorm(32 groups): per-channel sum (DVE reduce) + sum-of-squares (DVE
    scalar_tensor_tensor accum_out), group-reduce and per-channel broadcast
    via tiny group-indicator matmuls (fp32), fused apply h = xs*A + B'' with
    A = gamma*rsig, B'' = beta - (mu + bo_eff)*A (h computed from xsb = x +
    bo_eff so the residual-plus-bias tile is reused; the -bo_eff*A term
    compensates). h is written as fp8e4 in the DoubleRow (chunk-pair) layout.
  - q', k' = 16*(Wq h + bq), 16*Wk h: fp8 DoubleRow matmuls, PSUM->SBUF on
    DVE (bias for q only), stored zero-padded [64, 2, 1024] per head for the
    DoubleRow sim matmuls.
  - vT' = 16 * h^T Wv^T, fp8 DoubleRow, interleaved into the stationary
    [vA | ones | ones | vB] layout so one out-matmul yields numerator rows
    and softmax-denominator rows together.
  - Attention per head: simT' = k'^T q' = 256*simT (one DR matmul per
    512-col slice); E = exp(simT'/2048 - 3) on ACT, written fp8 directly
    (range <= ~134 < e4m3 max 240; the -3 shift cancels in num/den);
    numerator+denominator via DoubleRow out-matmuls over j-chunk pairs;
    att' = 16*att = num * reciprocal(den) (DVE), stored fp8.
  - Out-proj: fp8 DoubleRow (Wo x16) -> psum = 256*(Wo att); epilogue
    osb = psum*(1/256) + xsb on one DVE scalar_tensor_tensor; DMA out.

chain=K builds K dependent copies of the pipeline bounced through an internal
DRAM buffer (out_i -> x_{i+1}) for slope-based hardware timing.
"""

import os
import sys

if "/opt/trn_rl_repo" not in sys.path:
    sys.path.insert(0, "/opt/trn_rl_repo")
os.environ.setdefault("JAX_PLATFORMS", "axon,cpu")

import numpy as np

B = 8
C = 512
N = 1024
HEADS = 8
DH = 64
GROUPS = 32
EPS = 1e-6
WS = 16.0          # host-side weight scale (power of 2)
SCALE = 0.125      # DH ** -0.5
EXP_SCALE = SCALE / (WS * WS)   # exact power of 2: 2^-11
EXP_BIAS = -3.0
N_CORES = 8

_CACHE = {}


def _build(chain=1):
    import concourse.bacc as bacc
    import concourse.tile as tile
    from concourse import mybir
    import concourse.bass as bass
    from contextlib import ExitStack

    f32 = mybir.dt.float32
    u32 = mybir.dt.uint32
    f8 = mybir.dt.float8e4
    AF = mybir.ActivationFunctionType
    OP = mybir.AluOpType
    AX = mybir.AxisListType
    DR = mybir.MatmulPerfMode.DoubleRow

    nc = bacc.Bacc("TRN2", target_bir_lowering=False, debug=False,
                   num_devices=N_CORES)

    x_d = nc.dram_tensor("x", [C, N], f32, kind="ExternalInput").ap()
    # fp8 weights, host-prescaled x16, DoubleRow chunk-pair layout:
    # wq8[pass][p, i, m] = 16*w_qkvT[256*pass + 128*i + p, m]
    wqA_d = nc.dram_tensor("wq8A", [128, 2, 3 * C], f8, kind="ExternalInput").ap()
    wqB_d = nc.dram_tensor("wq8B", [128, 2, 3 * C], f8, kind="ExternalInput").ap()
    woA_d = nc.dram_tensor("wo8A", [128, 2, C], f8, kind="ExternalInput").ap()
    woB_d = nc.dram_tensor("wo8B", [128, 2, C], f8, kind="ExternalInput").ap()
    # fp8 residuals (w*16 - float(fp8(w*16))), unscaled so they accumulate in
    # the same PSUM group; halve the weight-quantization error at 0.5c/row.
    wqAl_d = nc.dram_tensor("wq8Alo", [128, 2, 3 * C], f8, kind="ExternalInput").ap()
    wqBl_d = nc.dram_tensor("wq8Blo", [128, 2, 3 * C], f8, kind="ExternalInput").ap()
    woAl_d = nc.dram_tensor("wo8Alo", [128, 2, C], f8, kind="ExternalInput").ap()
    woBl_d = nc.dram_tensor("wo8Blo", [128, 2, C], f8, kind="ExternalInput").ap()
    bq_d = nc.dram_tensor("bq16", [128, 4], f32, kind="ExternalInput").ap()
    bo_d = nc.dram_tensor("bo_eff", [128, 4], f32, kind="ExternalInput").ap()
    gam_d = nc.dram_tensor("gammaT", [128, 4], f32, kind="ExternalInput").ap()
    bet_d = nc.dram_tensor("betaT", [128, 4], f32, kind="ExternalInput").ap()
    gh_d = nc.dram_tensor("Ghat", [C, GROUPS], f32, kind="ExternalInput").ap()
    gt_d = nc.dram_tensor("GT", [GROUPS, C], f32, kind="ExternalInput").ap()
    out_d = nc.dram_tensor("out", [C, N], f32, kind="ExternalOutput").ap()
    bounce = nc.dram_tensor("chainbuf", [C, N], f32).ap() if chain > 1 else None

    with tile.TileContext(nc) as tc:
        with ExitStack() as ctx, nc.allow_low_precision(
                reason="fp8 matmul operands are quantized by design"):
            pers = ctx.enter_context(tc.tile_pool(name="pers", bufs=1))
            consts = ctx.enter_context(tc.tile_pool(name="consts", bufs=1))
            ep = ctx.enter_context(tc.tile_pool(name="ep", bufs=3))
            smalls = ctx.enter_context(tc.tile_pool(name="smalls", bufs=4))
            scrp = ctx.enter_context(tc.tile_pool(name="scrp", bufs=2))
            rcpp = ctx.enter_context(tc.tile_pool(name="rcpp", bufs=2))
            # PSUM: simp 2x[128,1024] (4 banks) + poutp 2x[128,1024] (4 banks)
            simp = ctx.enter_context(
                tc.tile_pool(name="simp", bufs=2, space="PSUM"))
            poutp = ctx.enter_context(
                tc.tile_pool(name="poutp", bufs=2, space="PSUM"))

            # ---- persistent weights / constants (loaded once) ----
            wqA = pers.tile([128, 2, 3 * C], f8, name="NM_wqA")
            wqB = pers.tile([128, 2, 3 * C], f8, name="NM_wqB")
            woA = pers.tile([128, 2, C], f8, name="NM_woA")
            woB = pers.tile([128, 2, C], f8, name="NM_woB")
            wqAl = pers.tile([128, 2, 3 * C], f8, name="NM_wqAl")
            wqBl = pers.tile([128, 2, 3 * C], f8, name="NM_wqBl")
            woAl = pers.tile([128, 2, C], f8, name="NM_woAl")
            woBl = pers.tile([128, 2, C], f8, name="NM_woBl")
            gh_sb = [consts.tile([128, GROUPS], f32, name=f"NM_gh{t}") for t in range(4)]
            gt_sb = consts.tile([GROUPS, C], f32, name="gt", tag="gt")
            bq_sb = consts.tile([128, 4], f32, name="bq", tag="bq")
            bo_sb = consts.tile([128, 4], f32, name="bo", tag="bo")
            gam_sb = consts.tile([128, 4], f32, name="gam", tag="gam")
            bet_sb = consts.tile([128, 4], f32, name="bet", tag="bet")
            magic_sb = consts.tile([GROUPS, 1], u32, name="magic", tag="magic")
            ebias_sb = consts.tile([128, 1], f32, name="ebias", tag="ebias")

            for t in range(4):
                nc.scalar.dma_start(out=gh_sb[t], in_=gh_d[t * 128:(t + 1) * 128, :])
            nc.scalar.dma_start(out=gt_sb, in_=gt_d)
            nc.scalar.dma_start(out=bq_sb, in_=bq_d)
            nc.scalar.dma_start(out=bo_sb, in_=bo_d)
            nc.scalar.dma_start(out=gam_sb, in_=gam_d)
            nc.scalar.dma_start(out=bet_sb, in_=bet_d)
            nc.gpsimd.dma_start(out=wqA, in_=wqA_d)
            nc.gpsimd.dma_start(out=wqB, in_=wqB_d)
            nc.gpsimd.dma_start(out=woA, in_=woA_d)
            nc.gpsimd.dma_start(out=woB, in_=woB_d)
            nc.gpsimd.dma_start(out=wqAl, in_=wqAl_d)
            nc.gpsimd.dma_start(out=wqBl, in_=wqBl_d)
            nc.gpsimd.dma_start(out=woAl, in_=woAl_d)
            nc.gpsimd.dma_start(out=woBl, in_=woBl_d)
            nc.vector.memset(magic_sb, 0x5f3759df)
            nc.vector.memset(ebias_sb, EXP_BIAS)

            # Persistent stationary tile for the attention out-matmuls:
            # vtt[:, j, 256*p + :] = [vT_A | ones64 | ones64 | vT_B] for head
            # pair p. Ones written once via memset; vT columns written per
            # iteration from the DoubleRow vT matmul.
            vtt = pers.tile([128, 8, N], f8, name="vtt", tag="vtt")
            nc.vector.memset(vtt, 1.0)

            # q/k fp8 tiles, zero-padded second K-tile for DoubleRow sim.
            # Layout [128, 2, 1024]: [:, 0, :] = values, [:, 1, :] = 0.
            qf8 = [pers.tile([128, 2, N], f8, name=f"NM_qf8_{m}") for m in range(4)]
            kf8 = [pers.tile([128, 2, N], f8, name=f"NM_kf8_{m}") for m in range(4)]
            for m in range(4):
                nc.vector.memset(qf8[m][:, 1, :], 0.0)
                nc.vector.memset(kf8[m][:, 1, :], 0.0)

            def body(it, x_src, dst):
                xs = [pers.tile([128, N], f32, name=f"xs{t}_{it}", tag=f"xs{t}")
                      for t in range(4)]
                xsb = [pers.tile([128, N], f32, name=f"xsb{t}_{it}", tag=f"xsb{t}")
                       for t in range(4)]
                # h fp8 in DoubleRow chunk-pair layout: hs01[:, i, :] = chunk i
                hs01 = pers.tile([128, 2, N], f8, name=f"hs01_{it}", tag="hs01")
                hs23 = pers.tile([128, 2, N], f8, name=f"hs23_{it}", tag="hs23")
                att01 = pers.tile([128, 2, N], f8, name=f"att01_{it}", tag="att01")
                att23 = pers.tile([128, 2, N], f8, name=f"att23_{it}", tag="att23")
                osb = [pers.tile([128, N], f32, name=f"osb{t}_{it}", tag=f"osb{t}")
                       for t in range(4)]
                sq_sb = consts.tile([128, 8], f32, name=f"sq_{it}", tag="sq")
                AB_sb = consts.tile([128, 8], f32, name=f"AB_{it}", tag="AB")
                musig = consts.tile([GROUPS, 2], f32, name=f"musig_{it}", tag="musig")

                for t in range(4):
                    nc.sync.dma_start(out=xs[t], in_=x_src[t * 128:(t + 1) * 128, :])

                # ---------------- GroupNorm ----------------
                for t in range(4):
                    nc.vector.reduce_sum(out=sq_sb[:, 2 * t:2 * t + 1],
                                         in_=xs[t], axis=AX.X)
                    scr = scrp.tile([128, N], f32, name=f"scr_{it}_{t}",
                                    tag="scr")
                    nc.vector.scalar_tensor_tensor(
                        out=scr, in0=xs[t], scalar=1.0, in1=xs[t],
                        op0=OP.mult, op1=OP.mult,
                        accum_out=sq_sb[:, 2 * t + 1:2 * t + 2])
                gstat = simp.tile([GROUPS, 2], f32, name=f"gstat_{it}",
                                  tag="sim")
                for t in range(4):
                    nc.tensor.matmul(gstat, lhsT=gh_sb[t],
                                     rhs=sq_sb[:, 2 * t:2 * t + 2],
                                     start=(t == 0), stop=(t == 3))
                # musig[:,0] = mu ; musig[:,1] = rsqrt(var + eps) via the
                # inverse-sqrt bit trick + 2 Newton steps (all DVE, keeps
                # ACT's table pinned on Exp).
                nc.vector.tensor_copy(out=musig[:, 0:1], in_=gstat[:, 0:1])
                musq = smalls.tile([GROUPS, 1], f32, name=f"musq_{it}",
                                   tag="musq")
                nc.vector.tensor_tensor(out=musq, in0=gstat[:, 0:1],
                                        in1=musig[:, 0:1], op=OP.mult)
                vpe = smalls.tile([GROUPS, 1], f32, name=f"vpe_{it}", tag="vpe")
                nc.vector.tensor_scalar(out=vpe, in0=musq, scalar1=-1.0,
                                        scalar2=EPS, op0=OP.mult, op1=OP.add)
                nc.vector.tensor_tensor(out=vpe, in0=vpe, in1=gstat[:, 1:2],
                                        op=OP.add)
                yib = smalls.tile([GROUPS, 1], u32, name=f"yib_{it}", tag="yib")
                nc.vector.tensor_scalar(out=yib, in0=vpe.bitcast(u32),
                                        scalar1=1, scalar2=None,
                                        op0=OP.logical_shift_right)
                nc.vector.tensor_tensor(out=yib, in0=magic_sb, in1=yib,
                                        op=OP.subtract)
                y = yib.bitcast(f32)
                t1 = smalls.tile([GROUPS, 1], f32, name=f"t1_{it}", tag="t1")
                for _ in range(2):
                    nc.vector.tensor_tensor(out=t1, in0=y, in1=y, op=OP.mult)
                    nc.vector.tensor_tensor(out=t1, in0=t1, in1=vpe, op=OP.mult)
                    nc.vector.tensor_scalar(out=t1, in0=t1, scalar1=-0.5,
                                            scalar2=1.5, op0=OP.mult, op1=OP.add)
                    nc.vector.tensor_tensor(out=y, in0=y, in1=t1, op=OP.mult)
                nc.vector.tensor_copy(out=musig[:, 1:2], in_=y)
                for t in range(4):
                    bcs = simp.tile([128, 2], f32, name=f"bcs_{it}_{t}",
                                    tag="sim")
                    nc.tensor.matmul(bcs, lhsT=gt_sb[:, t * 128:(t + 1) * 128],
                                     rhs=musig, start=True, stop=True)
                    # A = gamma * rsig_c ; B'' = beta - (mu_c + bo)*A
                    nc.vector.tensor_tensor(out=AB_sb[:, 2 * t:2 * t + 1],
                                            in0=bcs[:, 1:2],
                                            in1=gam_sb[:, t:t + 1], op=OP.mult)
                    mubo = smalls.tile([128, 1], f32, name=f"mubo_{it}_{t}",
                                       tag="mubo")
                    nc.vector.tensor_tensor(out=mubo, in0=bcs[:, 0:1],
                                            in1=bo_sb[:, t:t + 1], op=OP.add)
                    nc.vector.tensor_tensor(out=mubo, in0=mubo,
                                            in1=AB_sb[:, 2 * t:2 * t + 1],
                                            op=OP.mult)
                    nc.vector.tensor_tensor(out=AB_sb[:, 2 * t + 1:2 * t + 2],
                                            in0=bet_sb[:, t:t + 1], in1=mubo,
                                            op=OP.subtract)
                    nc.gpsimd.tensor_scalar_add(out=xsb[t], in0=xs[t],
                                                scalar1=bo_sb[:, t:t + 1])
                    hdst = hs01[:, t, :] if t < 2 else hs23[:, t - 2, :]
                    nc.vector.tensor_scalar(out=hdst, in0=xsb[t],
                                            scalar1=AB_sb[:, 2 * t:2 * t + 1],
                                            scalar2=AB_sb[:, 2 * t + 1:2 * t + 2],
                                            op0=OP.mult, op1=OP.add)

                # ---------------- q/k chunks (fp8 DoubleRow) ----------------
                def qk_chunk(m):
                    # m in 0..3 -> q chunk m ; m in 4..7 -> k chunk m-4
                    ps = simp.tile([128, N], f32, name=f"mmps_{it}_{m}", tag="sim")
                    msl = slice(m * 128, (m + 1) * 128)
                    for i2 in range(2):
                        isl = slice(i2 * 512, (i2 + 1) * 512)
                        nc.tensor.matmul(ps[:, isl], lhsT=wqA[:, :, msl],
                                         rhs=hs01[:, :, isl],
                                         start=True, stop=False, perf_mode=DR)
                        nc.tensor.matmul(ps[:, isl], lhsT=wqB[:, :, msl],
                                         rhs=hs23[:, :, isl],
                                         start=False, stop=False, perf_mode=DR)
                        nc.tensor.matmul(ps[:, isl], lhsT=wqAl[:, :, msl],
                                         rhs=hs01[:, :, isl],
                                         start=False, stop=False, perf_mode=DR)
                        nc.tensor.matmul(ps[:, isl], lhsT=wqBl[:, :, msl],
                                         rhs=hs23[:, :, isl],
                                         start=False, stop=True, perf_mode=DR)
                    if m < 4:
                        nc.vector.tensor_scalar_add(out=qf8[m][:, 0, :], in0=ps,
                                                    scalar1=bq_sb[:, m:m + 1])
                        # residual q_lo into the sim matmul's second K-tile
                        # (replaces the zero pad at zero PE cost)
                        nc.vector.scalar_tensor_tensor(
                            out=qf8[m][:, 1, :], in0=ps,
                            scalar=bq_sb[:, m:m + 1], in1=qf8[m][:, 0, :],
                            op0=OP.add, op1=OP.subtract)
                    else:
                        nc.vector.tensor_copy(out=kf8[m - 4][:, 0, :], in_=ps)
                        # duplicate k into the second K-tile (pairs with q_lo)
                        nc.gpsimd.tensor_copy(out=kf8[m - 4][:, 1, :],
                                              in_=kf8[m - 4][:, 0, :])

                def vt_chunk(j):
                    # vT'[n_j, vcol] for all heads, j-chunk j (128 rows of n)
                    jsl = slice(j * 128, (j + 1) * 128)
                    tps = simp.tile([128, 512], f32, name=f"vtp_{it}_{j}",
                                    tag="sim")
                    nc.tensor.matmul(tps, lhsT=hs01[:, :, jsl],
                                     rhs=wqA[:, :, 1024:1536],
                                     start=True, stop=False, perf_mode=DR)
                    nc.tensor.matmul(tps, lhsT=hs23[:, :, jsl],
                                     rhs=wqB[:, :, 1024:1536],
                                     start=False, stop=False, perf_mode=DR)
                    nc.tensor.matmul(tps, lhsT=hs01[:, :, jsl],
                                     rhs=wqAl[:, :, 1024:1536],
                                     start=False, stop=False, perf_mode=DR)
                    nc.tensor.matmul(tps, lhsT=hs23[:, :, jsl],
                                     rhs=wqBl[:, :, 1024:1536],
                                     start=False, stop=True, perf_mode=DR)
                    vbase = vtt[:, j, 0:1]
                    vdst = bass.AP(tensor=vbase.tensor, offset=vbase.offset,
                                   ap=[vbase.ap[0], [256, 4], [192, 2], [1, 64]])
                    nc.vector.tensor_copy(
                        out=vdst,
                        in_=tps.rearrange("p (q h d) -> p q h d", q=4, h=2))

                qk_chunk(0)
                qk_chunk(4)
                vt_chunk(0)
                vt_chunk(1)

                # ---------------- attention (per head pair p, head hh) ----------------
                for p in range(4):
                    qt, kt = qf8[p], kf8[p]
                    for hh in range(2):
                        hsl = slice(hh * 64, (hh + 1) * 64)
                        pout = poutp.tile([128, N], f32,
                                          name=f"pout_{it}_{p}_{hh}", tag="pout")
                        Et = None
                        for j in range(8):
                            ps = simp.tile([128, N], f32,
                                           name=f"sps_{it}_{p}_{hh}_{j}", tag="sim")
                            for i2 in range(2):
                                isl = slice(i2 * 512, (i2 + 1) * 512)
                                nc.tensor.matmul(
                                    ps[:, isl],
                                    lhsT=kt[hsl, :, j * 128:(j + 1) * 128],
                                    rhs=qt[hsl, :, isl],
                                    start=True, stop=True, perf_mode=DR)
                            if j % 2 == 0:
                                Et = ep.tile([128, 2, N], f8,
                                             name=f"E_{it}_{p}_{hh}_{j}", tag="E")
                            nc.scalar.activation(out=Et[:, j % 2, :], in_=ps,
                                                 func=AF.Exp, scale=EXP_SCALE,
                                                 bias=ebias_sb)
                            if p == 0 and hh == 0 and j >= 1 and j < 7:
                                # vT chunk j+1 just in time for its out-matmul
                                vt_chunk(j + 1)
                            if hh == 0 and 3 <= j <= 6:
                                # prefetch next pair's q/k under the exp stream
                                if p < 3:
                                    if j == 3:
                                        qk_chunk(p + 1)
                                    elif j == 5:
                                        qk_chunk(4 + p + 1)
                            if j % 2 == 1:
                                jj = j // 2
                                vb = vtt[:, j - 1,
                                         256 * p + hh * 128:256 * p + hh * 128 + 1]
                                vl = bass.AP(tensor=vb.tensor, offset=vb.offset,
                                             ap=[vb.ap[0], [N, 2], [1, 128]])
                                for i2 in range(2):
                                    isl = slice(i2 * 512, (i2 + 1) * 512)
                                    nc.tensor.matmul(
                                        pout[:, isl],
                                        lhsT=vl,
                                        rhs=Et[:, :, isl],
                                        start=(jj == 0), stop=(jj == 3),
                                        perf_mode=DR)
                        # epilogue: att' = 16*att = num * 1/den (fp8 out)
                        asl = slice(hh * 64, (hh + 1) * 64)
                        dsl = slice((1 - hh) * 64, (2 - hh) * 64)
                        rcp = rcpp.tile([128, N], f32, name=f"rcp_{it}_{p}_{hh}",
                                        tag="rcp")
                        nc.vector.reciprocal(out=rcp[asl, :], in_=pout[dsl, :])
                        adst = (att01 if p < 2 else att23)[asl.start:asl.stop,
                                                           p % 2, :]
                        nc.vector.tensor_tensor(out=adst, in0=pout[asl, :],
                                                in1=rcp[asl, :], op=OP.mult)

                # ---------------- out projection + bias + residual ----------------
                for t in range(4):
                    ps = simp.tile([128, N], f32, name=f"prps_{it}_{t}", tag="sim")
                    tsl = slice(t * 128, (t + 1) * 128)
                    for i2 in range(2):
                        isl = slice(i2 * 512, (i2 + 1) * 512)
                        nc.tensor.matmul(ps[:, isl], lhsT=woA[:, :, tsl],
                                         rhs=att01[:, :, isl],
                                         start=True, stop=False, perf_mode=DR)
                        nc.tensor.matmul(ps[:, isl], lhsT=woB[:, :, tsl],
                                         rhs=att23[:, :, isl],
                                         start=False, stop=False, perf_mode=DR)
                        nc.tensor.matmul(ps[:, isl], lhsT=woAl[:, :, tsl],
                                         rhs=att01[:, :, isl],
                                         start=False, stop=False, perf_mode=DR)
                        nc.tensor.matmul(ps[:, isl], lhsT=woBl[:, :, tsl],
                                         rhs=att23[:, :, isl],
                                         start=False, stop=True, perf_mode=DR)
                    nc.vector.scalar_tensor_tensor(
                        out=osb[t], in0=ps,
                        scalar=1.0 / (WS * WS), in1=xsb[t],
                        op0=OP.mult, op1=OP.add)
                    oeng = nc.gpsimd if t % 2 == 0 else nc.sync
                    oeng.dma_start(out=dst[t * 128:(t + 1) * 128, :],
                                   in_=osb[t])

            for it in range(chain):
                x_src = x_d if it == 0 else bounce
                dst = out_d if it == chain - 1 else bounce
                body(it, x_src, dst)

    nc.compile()
    return nc


def _get_nc(chain=1):
    key = ("nc", chain)
    if key not in _CACHE:
        _CACHE[key] = _build(chain)
    return _CACHE[key]


def _prep_inputs(x, gn_gamma, gn_beta, w_qkv, b_qkv, w_out, b_out):
    import ml_dtypes
    f = np.float32
    f8 = ml_dtypes.float8_e4m3
    xr = np.ascontiguousarray(np.asarray(x).reshape(B, C, N).astype(f))
    wqT = np.asarray(w_qkv).astype(f).T * WS          # [512, 1536], x16
    woT = np.asarray(w_out).astype(f).T * WS          # [512, 512], x16

    def dr_pack(w):  # [512, M] -> pass A/B fp8 [128, 2, M] + fp8 residuals
        wr = w.reshape(4, 128, -1)
        A = np.ascontiguousarray(np.stack([wr[0], wr[1]], 1))
        Bp = np.ascontiguousarray(np.stack([wr[2], wr[3]], 1))
        A8, B8 = A.astype(f8), Bp.astype(f8)
        Al8 = (A - A8.astype(np.float32)).astype(f8)
        Bl8 = (Bp - B8.astype(np.float32)).astype(f8)
        return A8, B8, Al8, Bl8

    wqA, wqB, wqAl, wqBl = dr_pack(wqT)
    woA, woB, woAl, woBl = dr_pack(woT)
    bq16 = np.ascontiguousarray(
        (np.asarray(b_qkv).astype(f)[0:512] * WS).reshape(4, 128).T)
    # v-bias folds into the out-projection bias exactly (softmax rows sum
    # to 1): out = W_o(att_nb + b_v) + b_out = W_o att_nb + (b_out + W_o b_v)
    bo_eff = (np.asarray(b_out).astype(np.float64) +
              np.asarray(w_out).astype(np.float64)
              @ np.asarray(b_qkv).astype(np.float64)[1024:1536]).astype(f)
    boT = np.ascontiguousarray(bo_eff.reshape(4, 128).T)
    gamT = np.ascontiguousarray(np.asarray(gn_gamma).astype(f).reshape(4, 128).T)
    betT = np.ascontiguousarray(np.asarray(gn_beta).astype(f).reshape(4, 128).T)
    ch = np.arange(C)
    Ghat = np.zeros((C, GROUPS), f)
    Ghat[ch, ch // 16] = 1.0 / (16 * N)
    GT = np.zeros((GROUPS, C), f)
    GT[ch // 16, ch] = 1.0
    shared = dict(wq8A=wqA, wq8B=wqB, wo8A=woA, wo8B=woB,
                  wq8Alo=wqAl, wq8Blo=wqBl, wo8Alo=woAl, wo8Blo=woBl,
                  bq16=bq16, bo_eff=boT, gammaT=gamT, betaT=betT,
                  Ghat=Ghat, GT=GT)
    return [dict(x=xr[i], **shared) for i in range(N_CORES)]


def _run(inputs, trace=False, trace_kwargs=None, chain=1):
    from concourse.bass_utils import run_bass_kernel_spmd
    nc = _get_nc(chain)
    in_maps = _prep_inputs(**inputs)
    res = run_bass_kernel_spmd(nc, in_maps, list(range(N_CORES)),
                               trace=trace, **(trace_kwargs or {}))
    out = np.stack([res.results[i]["out"] for i in range(N_CORES)])
    return out.reshape(B, C, 32, 32), res


def kernel(**inputs):
    out, _ = _run(inputs, trace=False)
    return out.astype(np.float32)


# revision 32
# speedup vs baseline: 3.3017x; 3.3017x over previous
"""Trainium2 Bass kernel for nn_Attention (GroupNorm + 1x1-conv QKV + MHA + out-proj + residual).

Sharding: data-parallel over batch — b=8 maps one batch element per NeuronCore (8 cores).
Weights are broadcast to all cores. No collectives.

All matmuls run as fp8e4 DoubleRow (0.5 cycles/row, two 128-row K-tiles per
pass = 4x the fp32r matmul throughput). Weights are pre-scaled x16 on the
host so their values sit in e4m3's normal range; the scale is compensated by
exact power-of-2 folds (exp scale /256, output epilogue x1/256). The q/k and
out-proj weight quantization error is halved by fp8 residual passes (w*16 -
fp8(w*16), accumulated into the same PSUM group at 0.5c/row). The d=64
attention matmuls get DoubleRow by zero-padding the second K-tile (PE cost
depends only on the output free size, so the zeros are free). The k-bias
cancels in softmax and the v-bias is folded into the out-proj bias (host), so
only the q-bias is applied on-device. GroupNorm's rsqrt runs on DVE via the
inverse-sqrt bit trick + 2 Newton steps, keeping ACT's table pinned to Exp
(Sqrt lives in a different ACT table; switching costs 2x 1.28us per pass);
the per-channel sum rides the xsb = x + bo_eff bias-add via accum_out, with
the group-mean corrected by a host constant. ACT then runs only the 64 exp
instructions (~54us) and paces the whole pipeline.

Per-core pipeline (one batch element, x: [512 ch, 1024 spatial]):
  - GroupNorm(32 groups): per-channel sum (DVE reduce) + sum-of-squares (DVE
    scalar_tensor_tensor accum_out), group-reduce and per-channel broadcast
    via tiny group-indicator matmuls (fp32), fused apply h = xs*A + B'' with
    A = gamma*rsig, B'' = beta - (mu + bo_eff)*A (h computed from xsb = x +
    bo_eff so the residual-plus-bias tile is reused; the -bo_eff*A term
    compensates). h is written as fp8e4 in the DoubleRow (chunk-pair) layout.
  - q', k' = 16*(Wq h + bq), 16*Wk h: fp8 DoubleRow matmuls, PSUM->SBUF on
    DVE (bias for q only), stored zero-padded [64, 2, 1024] per head for the
    DoubleRow sim matmuls.
  - vT' = 16 * h^T Wv^T, fp8 DoubleRow, interleaved into the stationary
    [vA | ones | ones | vB] layout so one out-matmul yields numerator rows
    and softmax-denominator rows together.
  - Attention per head: simT' = k'^T q' = 256*simT (one DR matmul per
    512-col slice); E = exp(simT'/2048 - 3) on ACT, written fp8 directly
    (range <= ~134 < e4m3 max 240; the -3 shift cancels in num/den);
    numerator+denominator via DoubleRow out-matmuls over j-chunk pairs;
    att' = 16*att = num * reciprocal(den) (DVE), stored fp8.
  - Out-proj: fp8 DoubleRow (Wo x16) -> psum = 256*(Wo att); epilogue
    osb = psum*(1/256) + xsb on one DVE scalar_tensor_tensor; DMA out.

chain=K builds K dependent copies of the pipeline bounced through an internal
DRAM buffer (out_i -> x_{i+1}) for slope-based hardware timing.
"""

import os
import sys

if "/opt/trn_rl_repo" not in sys.path:
    sys.path.insert(0, "/opt/trn_rl_repo")
os.environ.setdefault("JAX_PLATFORMS", "axon,cpu")

import numpy as np

B = 8
C = 512
N = 1024
HEADS = 8
DH = 64
GROUPS = 32
EPS = 1e-6
WS = 16.0          # host-side weight scale (power of 2)
SCALE = 0.125      # DH ** -0.5
EXP_SCALE = SCALE / (WS * WS)   # exact power of 2: 2^-11
EXP_BIAS = -3.0
N_CORES = 8

# feature flags (bisection/tuning)
WLO = os.environ.get("K_WLO", "1") == "1"       # weight-residual passes (q,k)
VLO = os.environ.get("K_VLO", "0") == "1"       # weight-residual passes (v)
OLO = os.environ.get("K_OLO", "1") == "1"       # weight-residual passes (w_out)
QLO = os.environ.get("K_QLO", "0") == "1"       # q-residual in sim K-tile 2
POOL_XSB = os.environ.get("K_POOL_XSB", "0") == "1"
POOL_KDUP = os.environ.get("K_POOL_KDUP", "0") == "1"

_CACHE = {}


def _build(chain=1):
    import concourse.bacc as bacc
    import concourse.tile as tile
    from concourse import mybir
    import concourse.bass as bass
    from contextlib import ExitStack

    f32 = mybir.dt.float32
    u32 = mybir.dt.uint32
    f8 = mybir.dt.float8e4
    AF = mybir.ActivationFunctionType
    OP = mybir.AluOpType
    AX = mybir.AxisListType
    DR = mybir.MatmulPerfMode.DoubleRow

    nc = bacc.Bacc("TRN2", target_bir_lowering=False, debug=False,
                   num_devices=N_CORES)

    x_d = nc.dram_tensor("x", [C, N], f32, kind="ExternalInput").ap()
    # fp8 weights, host-prescaled x16, DoubleRow chunk-pair layout:
    # wq8[pass][p, i, m] = 16*w_qkvT[256*pass + 128*i + p, m]
    wqA_d = nc.dram_tensor("wq8A", [128, 2, 3 * C], f8, kind="ExternalInput").ap()
    wqB_d = nc.dram_tensor("wq8B", [128, 2, 3 * C], f8, kind="ExternalInput").ap()
    woA_d = nc.dram_tensor("wo8A", [128, 2, C], f8, kind="ExternalInput").ap()
    woB_d = nc.dram_tensor("wo8B", [128, 2, C], f8, kind="ExternalInput").ap()
    # fp8 residuals (w*16 - float(fp8(w*16))), unscaled so they accumulate in
    # the same PSUM group; halve the weight-quantization error at 0.5c/row.
    wqAl_d = nc.dram_tensor("wq8Alo", [128, 2, 3 * C], f8, kind="ExternalInput").ap()
    wqBl_d = nc.dram_tensor("wq8Blo", [128, 2, 3 * C], f8, kind="ExternalInput").ap()
    woAl_d = nc.dram_tensor("wo8Alo", [128, 2, C], f8, kind="ExternalInput").ap()
    woBl_d = nc.dram_tensor("wo8Blo", [128, 2, C], f8, kind="ExternalInput").ap()
    bq_d = nc.dram_tensor("bq16", [128, 4], f32, kind="ExternalInput").ap()
    bo_d = nc.dram_tensor("bo_eff", [128, 4], f32, kind="ExternalInput").ap()
    gam_d = nc.dram_tensor("gammaT", [128, 4], f32, kind="ExternalInput").ap()
    bet_d = nc.dram_tensor("betaT", [128, 4], f32, kind="ExternalInput").ap()
    gh_d = nc.dram_tensor("Ghat", [C, GROUPS], f32, kind="ExternalInput").ap()
    boavg_d = nc.dram_tensor("boavg", [GROUPS, 1], f32, kind="ExternalInput").ap()
    gt_d = nc.dram_tensor("GT", [GROUPS, C], f32, kind="ExternalInput").ap()
    out_d = nc.dram_tensor("out", [C, N], f32, kind="ExternalOutput").ap()
    bounce = nc.dram_tensor("chainbuf", [C, N], f32).ap() if chain > 1 else None

    with tile.TileContext(nc) as tc:
        with ExitStack() as ctx, nc.allow_low_precision(
                reason="fp8 matmul operands are quantized by design"):
            pers = ctx.enter_context(tc.tile_pool(name="pers", bufs=1))
            consts = ctx.enter_context(tc.tile_pool(name="consts", bufs=1))
            ep = ctx.enter_context(tc.tile_pool(name="ep", bufs=4))
            smalls = ctx.enter_context(tc.tile_pool(name="smalls", bufs=4))
            scrp = ctx.enter_context(tc.tile_pool(name="scrp", bufs=2))
            rcpp = ctx.enter_context(tc.tile_pool(name="rcpp", bufs=2))
            # PSUM: simp 2x[128,1024] (4 banks) + poutp 2x[128,1024] (4 banks)
            simp = ctx.enter_context(
                tc.tile_pool(name="simp", bufs=int(os.environ.get("K_SIMP", "2")), space="PSUM"))
            poutp = ctx.enter_context(
                tc.tile_pool(name="poutp", bufs=int(os.environ.get("K_POUT", "2")), space="PSUM"))

            # ---- persistent weights / constants (loaded once) ----
            wqA = pers.tile([128, 2, 3 * C], f8, name="NM_wqA")
            wqB = pers.tile([128, 2, 3 * C], f8, name="NM_wqB")
            woA = pers.tile([128, 2, C], f8, name="NM_woA")
            woB = pers.tile([128, 2, C], f8, name="NM_woB")
            wqAl = pers.tile([128, 2, 3 * C], f8, name="NM_wqAl")
            wqBl = pers.tile([128, 2, 3 * C], f8, name="NM_wqBl")
            woAl = pers.tile([128, 2, C], f8, name="NM_woAl")
            woBl = pers.tile([128, 2, C], f8, name="NM_woBl")
            gh_sb = [consts.tile([128, GROUPS], f32, name=f"NM_gh{t}") for t in range(4)]
            gt_sb = consts.tile([GROUPS, C], f32, name="gt", tag="gt")
            bq_sb = consts.tile([128, 4], f32, name="bq", tag="bq")
            bo_sb = consts.tile([128, 4], f32, name="bo", tag="bo")
            gam_sb = consts.tile([128, 4], f32, name="gam", tag="gam")
            bet_sb = consts.tile([128, 4], f32, name="bet", tag="bet")
            magic_sb = consts.tile([GROUPS, 1], u32, name="magic", tag="magic")
            boavg_sb = consts.tile([GROUPS, 1], f32, name="boavg", tag="boavg")
            ebias_sb = consts.tile([128, 1], f32, name="ebias", tag="ebias")

            for t in range(4):
                nc.scalar.dma_start(out=gh_sb[t], in_=gh_d[t * 128:(t + 1) * 128, :])
            nc.scalar.dma_start(out=gt_sb, in_=gt_d)
            nc.scalar.dma_start(out=boavg_sb, in_=boavg_d)
            nc.scalar.dma_start(out=bq_sb, in_=bq_d)
            nc.scalar.dma_start(out=bo_sb, in_=bo_d)
            nc.scalar.dma_start(out=gam_sb, in_=gam_d)
            nc.scalar.dma_start(out=bet_sb, in_=bet_d)
            nc.gpsimd.dma_start(out=wqA, in_=wqA_d)
            nc.gpsimd.dma_start(out=wqB, in_=wqB_d)
            nc.gpsimd.dma_start(out=woA, in_=woA_d)
            nc.gpsimd.dma_start(out=woB, in_=woB_d)
            nc.gpsimd.dma_start(out=wqAl, in_=wqAl_d)
            nc.gpsimd.dma_start(out=wqBl, in_=wqBl_d)
            nc.gpsimd.dma_start(out=woAl, in_=woAl_d)
            nc.gpsimd.dma_start(out=woBl, in_=woBl_d)
            nc.vector.memset(magic_sb, 0x5f3759df)
            nc.vector.memset(ebias_sb, EXP_BIAS)

            # Persistent stationary tile for the attention out-matmuls:
            # vtt[:, j, 256*p + :] = [vT_A | ones64 | ones64 | vT_B] for head
            # pair p. Ones written once via memset; vT columns written per
            # iteration from the DoubleRow vT matmul.
            vtt = pers.tile([128, 8, N], f8, name="vtt", tag="vtt")
            nc.vector.memset(vtt, 1.0)

            # q/k fp8 tiles, zero-padded second K-tile for DoubleRow sim.
            # Layout [128, 2, 1024]: [:, 0, :] = values, [:, 1, :] = 0.
            qf8 = [pers.tile([128, 2, N], f8, name=f"NM_qf8_{m}") for m in range(4)]
            kf8 = [pers.tile([128, 2, N], f8, name=f"NM_kf8_{m}") for m in range(4)]
            for m in range(4):
                nc.vector.memset(qf8[m][:, 1, :], 0.0)
                nc.vector.memset(kf8[m][:, 1, :], 0.0)

            def body(it, x_src, dst):
                xs = [pers.tile([128, N], f32, name=f"xs{t}_{it}", tag=f"xs{t}")
                      for t in range(4)]
                xsb = [pers.tile([128, N], f32, name=f"xsb{t}_{it}", tag=f"xsb{t}")
                       for t in range(4)]
                # h fp8 in DoubleRow chunk-pair layout: hs01[:, i, :] = chunk i
                hs01 = pers.tile([128, 2, N], f8, name=f"hs01_{it}", tag="hs01")
                hs23 = pers.tile([128, 2, N], f8, name=f"hs23_{it}", tag="hs23")
                att01 = pers.tile([128, 2, N], f8, name=f"att01_{it}", tag="att01")
                att23 = pers.tile([128, 2, N], f8, name=f"att23_{it}", tag="att23")
                osb = [pers.tile([128, N], f32, name=f"osb{t}_{it}", tag=f"osb{t}")
                       for t in range(4)]
                sq_sb = consts.tile([128, 8], f32, name=f"sq_{it}", tag="sq")
                AB_sb = consts.tile([128, 8], f32, name=f"AB_{it}", tag="AB")
                musig = consts.tile([GROUPS, 2], f32, name=f"musig_{it}", tag="musig")

                for t in range(4):
                    nc.sync.dma_start(out=xs[t], in_=x_src[t * 128:(t + 1) * 128, :])

                # ---------------- GroupNorm ----------------
                # xsb = x + bo_eff doubles as the sum pass (accum_out);
                # the group-mean is corrected by the constant group-avg of
                # bo_eff after the group-reduce matmul.
                for t in range(4):
                    nc.vector.tensor_scalar(
                        out=xsb[t], in0=xs[t], scalar1=bo_sb[:, t:t + 1],
                        scalar2=0.0, op0=OP.add, op1=OP.add,
                        accum_out=sq_sb[:, 2 * t:2 * t + 1])
                    scr = scrp.tile([128, N], f32, name=f"scr_{it}_{t}",
                                    tag="scr")
                    nc.vector.scalar_tensor_tensor(
                        out=scr, in0=xs[t], scalar=1.0, in1=xs[t],
                        op0=OP.mult, op1=OP.mult,
                        accum_out=sq_sb[:, 2 * t + 1:2 * t + 2])
                gstat = simp.tile([GROUPS, 2], f32, name=f"gstat_{it}",
                                  tag="sim")
                for t in range(4):
                    nc.tensor.matmul(gstat, lhsT=gh_sb[t],
                                     rhs=sq_sb[:, 2 * t:2 * t + 2],
                                     start=(t == 0), stop=(t == 3))
                # musig[:,0] = mu ; musig[:,1] = rsqrt(var + eps) via the
                # inverse-sqrt bit trick + 2 Newton steps (all DVE, keeps
                # ACT's table pinned on Exp).
                nc.vector.tensor_tensor(out=musig[:, 0:1], in0=gstat[:, 0:1],
                                        in1=boavg_sb, op=OP.subtract)
                musq = smalls.tile([GROUPS, 1], f32, name=f"musq_{it}",
                                   tag="musq")
                nc.vector.tensor_tensor(out=musq, in0=musig[:, 0:1],
                                        in1=musig[:, 0:1], op=OP.mult)
                vpe = smalls.tile([GROUPS, 1], f32, name=f"vpe_{it}", tag="vpe")
                nc.vector.tensor_scalar(out=vpe, in0=musq, scalar1=-1.0,
                                        scalar2=EPS, op0=OP.mult, op1=OP.add)
                nc.vector.tensor_tensor(out=vpe, in0=vpe, in1=gstat[:, 1:2],
                                        op=OP.add)
                yib = smalls.tile([GROUPS, 1], u32, name=f"yib_{it}", tag="yib")
                nc.vector.tensor_scalar(out=yib, in0=vpe.bitcast(u32),
                                        scalar1=1, scalar2=None,
                                        op0=OP.logical_shift_right)
                nc.vector.tensor_tensor(out=yib, in0=magic_sb, in1=yib,
                                        op=OP.subtract)
                y = yib.bitcast(f32)
                t1 = smalls.tile([GROUPS, 1], f32, name=f"t1_{it}", tag="t1")
                for _ in range(2):
                    nc.vector.tensor_tensor(out=t1, in0=y, in1=y, op=OP.mult)
                    nc.vector.tensor_tensor(out=t1, in0=t1, in1=vpe, op=OP.mult)
                    nc.vector.tensor_scalar(out=t1, in0=t1, scalar1=-0.5,
                                            scalar2=1.5, op0=OP.mult, op1=OP.add)
                    nc.vector.tensor_tensor(out=y, in0=y, in1=t1, op=OP.mult)
                nc.vector.tensor_copy(out=musig[:, 1:2], in_=y)
                for t in range(4):
                    bcs = simp.tile([128, 2], f32, name=f"bcs_{it}_{t}",
                                    tag="sim")
                    nc.tensor.matmul(bcs, lhsT=gt_sb[:, t * 128:(t + 1) * 128],
                                     rhs=musig, start=True, stop=True)
                    # A = gamma * rsig_c ; B'' = beta - (mu_c + bo)*A
                    nc.vector.tensor_tensor(out=AB_sb[:, 2 * t:2 * t + 1],
                                            in0=bcs[:, 1:2],
                                            in1=gam_sb[:, t:t + 1], op=OP.mult)
                    mubo = smalls.tile([128, 1], f32, name=f"mubo_{it}_{t}",
                                       tag="mubo")
                    nc.vector.tensor_tensor(out=mubo, in0=bcs[:, 0:1],
                                            in1=bo_sb[:, t:t + 1], op=OP.add)
                    nc.vector.tensor_tensor(out=mubo, in0=mubo,
                                            in1=AB_sb[:, 2 * t:2 * t + 1],
                                            op=OP.mult)
                    nc.vector.tensor_tensor(out=AB_sb[:, 2 * t + 1:2 * t + 2],
                                            in0=bet_sb[:, t:t + 1], in1=mubo,
                                            op=OP.subtract)
                    hdst = hs01[:, t, :] if t < 2 else hs23[:, t - 2, :]
                    nc.vector.tensor_scalar(out=hdst, in0=xsb[t],
                                            scalar1=AB_sb[:, 2 * t:2 * t + 1],
                                            scalar2=AB_sb[:, 2 * t + 1:2 * t + 2],
                                            op0=OP.mult, op1=OP.add)

                # ---------------- q/k chunks (fp8 DoubleRow) ----------------
                def qk_chunk(m):
                    # m in 0..3 -> q chunk m ; m in 4..7 -> k chunk m-4
                    ps = simp.tile([128, N], f32, name=f"mmps_{it}_{m}", tag="sim")
                    msl = slice(m * 128, (m + 1) * 128)
                    passes = [(wqA, hs01), (wqB, hs23)]
                    if WLO:
                        passes += [(wqAl, hs01), (wqBl, hs23)]
                    for pi, (wt, ht) in enumerate(passes):
                        for i2 in range(2):
                            isl = slice(i2 * 512, (i2 + 1) * 512)
                            nc.tensor.matmul(ps[:, isl], lhsT=wt[:, :, msl],
                                             rhs=ht[:, :, isl],
                                             start=(pi == 0),
                                             stop=(pi == len(passes) - 1),
                                             perf_mode=DR)
                    if m < 4:
                        nc.vector.tensor_scalar_add(out=qf8[m][:, 0, :], in0=ps,
                                                    scalar1=bq_sb[:, m:m + 1])
                        if QLO:
                            nc.vector.scalar_tensor_tensor(
                                out=qf8[m][:, 1, :], in0=ps,
                                scalar=bq_sb[:, m:m + 1], in1=qf8[m][:, 0, :],
                                op0=OP.add, op1=OP.subtract)
                    else:
                        nc.vector.tensor_copy(out=kf8[m - 4][:, 0, :], in_=ps)
                        if QLO:
                            keng = nc.gpsimd if POOL_KDUP else nc.vector
                            keng.tensor_copy(out=kf8[m - 4][:, 1, :],
                                             in_=kf8[m - 4][:, 0, :])

                def vt_chunk(j):
                    # vT'[n_j, vcol] for all heads, j-chunk j (128 rows of n)
                    jsl = slice(j * 128, (j + 1) * 128)
                    tps = simp.tile([128, 512], f32, name=f"vtp_{it}_{j}",
                                    tag="sim")
                    vpasses = [(hs01, wqA), (hs23, wqB)]
                    if VLO:
                        vpasses += [(hs01, wqAl), (hs23, wqBl)]
                    for pi, (ht, wt) in enumerate(vpasses):
                        nc.tensor.matmul(tps, lhsT=ht[:, :, jsl],
                                         rhs=wt[:, :, 1024:1536],
                                         start=(pi == 0),
                                         stop=(pi == len(vpasses) - 1),
                                         perf_mode=DR)
                    vbase = vtt[:, j, 0:1]
                    vdst = bass.AP(tensor=vbase.tensor, offset=vbase.offset,
                                   ap=[vbase.ap[0], [256, 4], [192, 2], [1, 64]])
                    nc.vector.tensor_copy(
                        out=vdst,
                        in_=tps.rearrange("p (q h d) -> p q h d", q=4, h=2))

                qk_chunk(0)
                qk_chunk(4)

                # ---------------- attention (per head pair p, head hh) ----------------
                for p in range(4):
                    qt, kt = qf8[p], kf8[p]
                    for hh in range(2):
                        hsl = slice(hh * 64, (hh + 1) * 64)
                        pout = poutp.tile([128, N], f32,
                                          name=f"pout_{it}_{p}_{hh}", tag="pout")
                        Et = None
                        for j in range(8):
                            ps = simp.tile([128, N], f32,
                                           name=f"sps_{it}_{p}_{hh}_{j}", tag="sim")
                            for i2 in range(2):
                                isl = slice(i2 * 512, (i2 + 1) * 512)
                                nc.tensor.matmul(
                                    ps[:, isl],
                                    lhsT=kt[hsl, :, j * 128:(j + 1) * 128],
                                    rhs=qt[hsl, :, isl],
                                    start=True, stop=True, perf_mode=DR)
                            if j % 2 == 0:
                                Et = ep.tile([128, 2, N], f8,
                                             name=f"E_{it}_{p}_{hh}_{j}", tag="E")
                            nc.scalar.activation(out=Et[:, j % 2, :], in_=ps,
                                                 func=AF.Exp, scale=EXP_SCALE,
                                                 bias=ebias_sb)
                            if p == 0 and hh == 0 and j < 7:
                                # vT chunks just in time for their out-matmuls
                                if j == 0:
                                    vt_chunk(0)
                                    vt_chunk(1)
                                else:
                                    vt_chunk(j + 1)
                            if hh == 0 and p < 3:
                                # prefetch next pair's q/k under the exp stream
                                if j == 3:
                                    qk_chunk(p + 1)
                                elif j == 5:
                                    qk_chunk(4 + p + 1)
                            if j % 2 == 1:
                                jj = j // 2
                                vb = vtt[:, j - 1,
                                         256 * p + hh * 128:256 * p + hh * 128 + 1]
                                vl = bass.AP(tensor=vb.tensor, offset=vb.offset,
                                             ap=[vb.ap[0], [N, 2], [1, 128]])
                                for i2 in range(2):
                                    isl = slice(i2 * 512, (i2 + 1) * 512)
                                    nc.tensor.matmul(
                                        pout[:, isl],
                                        lhsT=vl,
                                        rhs=Et[:, :, isl],
                                        start=(jj == 0), stop=(jj == 3),
                                        perf_mode=DR)
                        # epilogue: att' = 16*att = num * 1/den (fp8 out)
                        asl = slice(hh * 64, (hh + 1) * 64)
                        dsl = slice((1 - hh) * 64, (2 - hh) * 64)
                        rcp = rcpp.tile([128, N], f32, name=f"rcp_{it}_{p}_{hh}",
                                        tag="rcp")
                        nc.vector.reciprocal(out=rcp[asl, :], in_=pout[dsl, :])
                        adst = (att01 if p < 2 else att23)[asl.start:asl.stop,
                                                           p % 2, :]
                        nc.vector.tensor_tensor(out=adst, in0=pout[asl, :],
                                                in1=rcp[asl, :], op=OP.mult)

                # ---------------- out projection + bias + residual ----------------
                for t in range(4):
                    ps = simp.tile([128, N], f32, name=f"prps_{it}_{t}", tag="sim")
                    tsl = slice(t * 128, (t + 1) * 128)
                    opasses = [(woA, att01), (woB, att23)]
                    if OLO:
                        opasses += [(woAl, att01), (woBl, att23)]
                    for pi, (wt, at) in enumerate(opasses):
                        for i2 in range(2):
                            isl = slice(i2 * 512, (i2 + 1) * 512)
                            nc.tensor.matmul(ps[:, isl], lhsT=wt[:, :, tsl],
                                             rhs=at[:, :, isl],
                                             start=(pi == 0),
                                             stop=(pi == len(opasses) - 1),
                                             perf_mode=DR)
                    nc.vector.scalar_tensor_tensor(
                        out=osb[t], in0=ps,
                        scalar=1.0 / (WS * WS), in1=xsb[t],
                        op0=OP.mult, op1=OP.add)
                    oeng = nc.gpsimd if t % 2 == 0 else nc.sync
                    oeng.dma_start(out=dst[t * 128:(t + 1) * 128, :],
                                   in_=osb[t])

            for it in range(chain):
                x_src = x_d if it == 0 else bounce
                dst = out_d if it == chain - 1 else bounce
                body(it, x_src, dst)

    nc.compile()
    return nc


def _get_nc(chain=1):
    key = ("nc", chain)
    if key not in _CACHE:
        _CACHE[key] = _build(chain)
    return _CACHE[key]


def _prep_inputs(x, gn_gamma, gn_beta, w_qkv, b_qkv, w_out, b_out):
    import ml_dtypes
    f = np.float32
    f8 = ml_dtypes.float8_e4m3
    xr = np.ascontiguousarray(np.asarray(x).reshape(B, C, N).astype(f))
    wqT = np.asarray(w_qkv).astype(f).T * WS          # [512, 1536], x16
    woT = np.asarray(w_out).astype(f).T * WS          # [512, 512], x16

    def dr_pack(w):  # [512, M] -> pass A/B fp8 [128, 2, M] + fp8 residuals
        wr = w.reshape(4, 128, -1)
        A = np.ascontiguousarray(np.stack([wr[0], wr[1]], 1))
        Bp = np.ascontiguousarray(np.stack([wr[2], wr[3]], 1))
        A8, B8 = A.astype(f8), Bp.astype(f8)
        Al8 = (A - A8.astype(np.float32)).astype(f8)
        Bl8 = (Bp - B8.astype(np.float32)).astype(f8)
        return A8, B8, Al8, Bl8

    wqA, wqB, wqAl, wqBl = dr_pack(wqT)
    woA, woB, woAl, woBl = dr_pack(woT)
    bq16 = np.ascontiguousarray(
        (np.asarray(b_qkv).astype(f)[0:512] * WS).reshape(4, 128).T)
    # v-bias folds into the out-projection bias exactly (softmax rows sum
    # to 1): out = W_o(att_nb + b_v) + b_out = W_o att_nb + (b_out + W_o b_v)
    bo_eff = (np.asarray(b_out).astype(np.float64) +
              np.asarray(w_out).astype(np.float64)
              @ np.asarray(b_qkv).astype(np.float64)[1024:1536]).astype(f)
    boT = np.ascontiguousarray(bo_eff.reshape(4, 128).T)
    boavg = np.ascontiguousarray(bo_eff.reshape(GROUPS, 16).mean(1, keepdims=True))
    gamT = np.ascontiguousarray(np.asarray(gn_gamma).astype(f).reshape(4, 128).T)
    betT = np.ascontiguousarray(np.asarray(gn_beta).astype(f).reshape(4, 128).T)
    ch = np.arange(C)
    Ghat = np.zeros((C, GROUPS), f)
    Ghat[ch, ch // 16] = 1.0 / (16 * N)
    GT = np.zeros((GROUPS, C), f)
    GT[ch // 16, ch] = 1.0
    shared = dict(wq8A=wqA, wq8B=wqB, wo8A=woA, wo8B=woB,
                  wq8Alo=wqAl, wq8Blo=wqBl, wo8Alo=woAl, wo8Blo=woBl,
                  bq16=bq16, bo_eff=boT, boavg=boavg, gammaT=gamT,
                  betaT=betT, Ghat=Ghat, GT=GT)
    return [dict(x=xr[i], **shared) for i in range(N_CORES)]


def _run(inputs, trace=False, trace_kwargs=None, chain=1):
    from concourse.bass_utils import run_bass_kernel_spmd
    nc = _get_nc(chain)
    in_maps = _prep_inputs(**inputs)
    res = run_bass_kernel_spmd(nc, in_maps, list(range(N_CORES)),
                               trace=trace, **(trace_kwargs or {}))
    out = np.stack([res.results[i]["out"] for i in range(N_CORES)])
    return out.reshape(B, C, 32, 32), res


def kernel(**inputs):
    out, _ = _run(inputs, trace=False)
    return out.astype(np.float32)
